# revision 1
# baseline (speedup 1.0000x reference)
"""Causal self-attention (B=2, T=2048, D=1024, H=16) on 8 trn2 cores.

Sharding: tensor-parallel over heads x data-parallel over batch.
Core c handles batch b = c // 4, head group g = c % 4 (heads 4g..4g+3).
Host pre-slices/pre-transposes weight+activation shards; each core
returns a partial y (its heads' contribution); host sums groups of 4.
"""

import os
import sys

for _p in ("/opt/trn_rl_repo", "/root/.axon_site/_ro/trn_rl_repo"):
    if os.path.isdir(_p) and _p not in sys.path:
        sys.path.insert(0, _p)

import numpy as np

import concourse.bass as bass
import concourse.mybir as mybir
import concourse.tile as tile
from concourse import bacc
from concourse.bass_utils import run_bass_kernel_spmd

F32 = mybir.dt.float32
F32R = mybir.dt.float32r

B, T, C = 2, 2048, 1024
NHEAD_TOT = 16
DH = 64
NCORES = 8
NH = 4          # heads per core
NPAIR = 2       # head pairs per core
CK = C // 128   # contraction chunks (8)
TT = 512        # attention t-tile width
NTT = T // TT   # 4
NSCH = T // 128  # s chunks (16)
FQK = 2 * NH * DH  # 512 cols of qkv^T for q+k
FV = NH * DH       # 256 cols for v


def r32(ap):
    return ap.bitcast(F32R)


def build_nc(dbg=False):
    nc = bacc.Bacc("TRN2", target_bir_lowering=False, debug=False)

    xT = nc.dram_tensor("xT", [C, T], F32R, kind="ExternalInput")
    wqkvT = nc.dram_tensor("wqkvT", [C, FQK + FV], F32R, kind="ExternalInput")
    woutT = nc.dram_tensor("woutT", [NH * DH, C], F32R, kind="ExternalInput")
    y = nc.dram_tensor("y", [T, C], F32, kind="ExternalOutput")
    L_dram = nc.dram_tensor("L_scratch", [2 * NPAIR * NTT, TT], F32)
    R_dram = nc.dram_tensor("R_scratch", [2 * NPAIR * NTT, TT], F32R)
    if dbg:
        dbg_qkT = nc.dram_tensor("dbg_qkT", [128, 4, T], F32R, kind="ExternalOutput")
        dbg_v = nc.dram_tensor("dbg_v", [128, NSCH, NH, DH + 1], F32R, kind="ExternalOutput")
        dbg_oT = nc.dram_tensor("dbg_oT", [128, NPAIR, T], F32R, kind="ExternalOutput")
        dbg_pt = nc.dram_tensor("dbg_pt", [128, 2, TT], F32R, kind="ExternalOutput")
        dbg_pv = nc.dram_tensor("dbg_pv", [DH + 1, TT], F32, kind="ExternalOutput")
        dbg_bc = nc.dram_tensor("dbg_bc", [128, TT], F32, kind="ExternalOutput")

    EXP = mybir.ActivationFunctionType.Exp

    with tile.TileContext(nc) as tc:
        with (
            tc.tile_pool(name="const", bufs=1) as const,
            tc.tile_pool(name="ptp", bufs=4) as ptp,
            tc.tile_pool(name="bcp", bufs=2) as bcp,
            tc.tile_pool(name="rcp", bufs=2) as rcp,
            tc.tile_pool(name="yp", bufs=2) as yp,
            tc.tile_pool(name="psA", bufs=2, space="PSUM") as psA,
            tc.tile_pool(name="psV", bufs=4, space="PSUM") as psV,
        ):
            # ---- persistent SBUF ----
            xT_sb = const.tile([128, CK, T], F32R)          # x^T  (c-major)
            wqkvT_sb = const.tile([128, CK, FQK + FV], F32R)  # W_qkv^T cols [q(4x64)|k(4x64)|v(4x64)]
            woutT_sb = const.tile([128, NPAIR, C], F32R)    # W_out^T rows per head pair
            qkT_sb = const.tile([128, 4, T], F32R)          # [qPair0|qPair1|kPair0|kPair1] x T
            v_sb = const.tile([128, NSCH, NH, DH + 1], F32R)  # V (s-major) + ones column
            oT_sb = const.tile([128, NPAIR, T], F32R)       # normalized O^T, pair-stacked

            for ci in range(CK):
                nc.sync.dma_start(xT_sb[:, ci, :], xT[ci * 128:(ci + 1) * 128, :])
                nc.sync.dma_start(wqkvT_sb[:, ci, :], wqkvT[ci * 128:(ci + 1) * 128, :])
            for pr in range(NPAIR):
                nc.sync.dma_start(woutT_sb[:, pr, :], woutT[pr * 128:(pr + 1) * 128, :])
            # 1.0f bit pattern; direct f32r memset is rejected by walrus codegen
            nc.vector.memset(v_sb[:, :, :, DH:DH + 1].bitcast(mybir.dt.uint32),
                             0x3F800000)
            ones1 = const.tile([1, 64], F32R)  # lhsT for recipL row->partition broadcast
            nc.vector.memset(ones1.bitcast(mybir.dt.uint32), 0x3F800000)

            # ---- QKV projection ----
            # q^T/k^T: psum[f128, t512] = sum_c wqkvT[c, f].T @ xT[c, t]
            for ft in range(4):
                for tt in range(NTT):
                    ps = psA.tile([128, 2, TT], F32)
                    for ci in range(CK):
                        nc.tensor.matmul(
                            ps[:, 0, :],
                            wqkvT_sb[:, ci, ft * 128:(ft + 1) * 128],
                            xT_sb[:, ci, tt * TT:(tt + 1) * TT],
                            start=(ci == 0), stop=(ci == CK - 1),
                        )
                    nc.vector.tensor_copy(qkT_sb[:, ft, tt * TT:(tt + 1) * TT], ps[:, 0, :])
            # v natural: psum[t128, f256] = xT[c, t].T @ wqkvT[c, v]
            for si in range(NSCH):
                ps = psA.tile([128, 2, TT], F32)
                for ci in range(CK):
                    nc.tensor.matmul(
                        ps[:, 0, 0:FV],
                        xT_sb[:, ci, si * 128:(si + 1) * 128],
                        wqkvT_sb[:, ci, FQK:FQK + FV],
                        start=(ci == 0), stop=(ci == CK - 1),
                    )
                nc.vector.tensor_copy(
                    v_sb[:, si, :, 0:DH],
                    ps[:, 0, 0:FV].rearrange("p (h d) -> p h d", h=NH),
                )

            # ---- attention (S^T orientation), per head pair ----
            # oT_sb holds UNNORMALIZED O^T during the loop; softmax sums (L)
            # are shipped to DRAM and normalization happens in a batched end
            # phase so PV psums are released by plain copies (no recip chain).
            for pr in range(NPAIR):
                for tt in range(NTT):
                    n_ss = 4 * (tt + 1)  # causal: s-chunks 0 .. 4*tt+3
                    pv = [psV.tile([DH + 1, TT], F32, tag="pv", name=f"pv{pr}_{tt}_{k}")
                          for k in range(2)]
                    for sq in range(n_ss // 2):
                        for hi in range(2):
                            h = pr * 2 + hi
                            ps = psA.tile([128, 2, TT], F32)
                            for i in range(2):
                                ss = 2 * sq + i
                                nc.tensor.matmul(
                                    ps[:, i, :],
                                    qkT_sb[hi * 64:(hi + 1) * 64, 2 + pr, ss * 128:(ss + 1) * 128],
                                    qkT_sb[hi * 64:(hi + 1) * 64, pr, tt * TT:(tt + 1) * TT],
                                )
                            pt = ptp.tile([128, 2, TT], F32R)
                            nc.scalar.activation(pt, ps, EXP, scale=0.125)
                            if 2 * sq >= 4 * tt:  # diagonal quad: zero where s > t
                                nc.gpsimd.affine_select(
                                    out=pt, in_=pt,
                                    compare_op=mybir.AluOpType.is_ge,
                                    fill=0.0,
                                    base=tt * TT - 2 * sq * 128,
                                    channel_multiplier=-1,
                                    pattern=[[-128, 2], [1, TT]],
                                )
                            if dbg and pr == 0 and tt == 0 and sq == 0 and hi == 0:
                                nc.sync.dma_start(dbg_pt[:], pt)
                            for i in range(2):
                                ss = 2 * sq + i
                                nc.tensor.matmul(
                                    pv[hi],
                                    v_sb[:, ss, h, :],
                                    pt[:, i, :],
                                    start=(ss == 0), stop=(ss == n_ss - 1),
                                )
                    if dbg and pr == 0 and tt == 0:
                        pvcpy = bcp.tile([DH + 1, TT], F32, tag="pvcpy")
                        nc.vector.tensor_copy(pvcpy, pv[0])
                        nc.sync.dma_start(dbg_pv[:], pvcpy)
                    for hi in range(2):
                        idx = (pr * NTT + tt) * 2 + hi
                        nc.vector.tensor_copy(
                            oT_sb[hi * 64:(hi + 1) * 64, pr, tt * TT:(tt + 1) * TT],
                            pv[hi][0:DH, :],
                        )
                        lrow = rcp.tile([1, TT], F32, tag="lrow", name=f"lrow{idx}")
                        nc.vector.tensor_copy(lrow, pv[hi][DH:DH + 1, :])
                        nc.sync.dma_start(L_dram[idx:idx + 1, :], lrow[0:1, :])

            # ---- batched softmax normalization ----
            # gather all 16 L rows as [128, 64], one fast reciprocal, ship
            # back, then per-tile outer-product broadcast + multiply.
            lsq = bcp.tile([128, 64], F32, tag="lsq")
            nc.sync.dma_start(lsq, L_dram[:, :].rearrange("r (s j) -> (r s) j", j=64))
            with nc.allow_low_precision("f32r recip feeds f32r matmul rhs"):
                rsq = bcp.tile([128, 64], F32R, tag="rsq")
                nc.vector.reciprocal(rsq, lsq)
            nc.sync.dma_start(R_dram[:, :].rearrange("r (s j) -> (r s) j", j=64), rsq)
            for pr in range(NPAIR):
                for tt in range(NTT):
                    bq = psA.tile([128, 2, TT], F32, tag="ps", name=f"bq{pr}_{tt}")
                    bc = bcp.tile([128, TT], F32)
                    for hi in range(2):
                        idx = (pr * NTT + tt) * 2 + hi
                        rcr = rcp.tile([1, TT], F32R, tag="rcr", name=f"rcr{idx}")
                        nc.sync.dma_start(rcr[0:1, :], R_dram[idx:idx + 1, :])
                        nc.tensor.matmul(bq[:, hi, :][0:64, :], ones1, rcr)
                        nc.vector.tensor_copy(
                            bc[hi * 64:(hi + 1) * 64, :], bq[:, hi, :][0:64, :]
                        )
                        nc.vector.tensor_mul(
                            oT_sb[hi * 64:(hi + 1) * 64, pr, tt * TT:(tt + 1) * TT],
                            oT_sb[hi * 64:(hi + 1) * 64, pr, tt * TT:(tt + 1) * TT],
                            bc[hi * 64:(hi + 1) * 64, :],
                        )
                    if dbg and pr == 0 and tt == 0:
                        nc.sync.dma_start(dbg_bc[:], bc)

            if dbg:
                nc.sync.dma_start(dbg_qkT[:], qkT_sb)
                nc.sync.dma_start(dbg_v[:], v_sb)
                nc.sync.dma_start(dbg_oT[:], oT_sb)

            # ---- output projection: y[t, o] = sum_pr oT[d, t].T @ woutT[d, o] ----
            for tq in range(T // 128):
                for ot in range(C // TT):
                    ps = psA.tile([128, 2, TT], F32)
                    for pr in range(NPAIR):
                        nc.tensor.matmul(
                            ps[:, 0, :],
                            oT_sb[:, pr, tq * 128:(tq + 1) * 128],
                            woutT_sb[:, pr, ot * TT:(ot + 1) * TT],
                            start=(pr == 0), stop=(pr == NPAIR - 1),
                        )
                    yt = yp.tile([128, TT], F32)
                    nc.vector.tensor_copy(yt, ps[:, 0, :])
                    nc.sync.dma_start(y[tq * 128:(tq + 1) * 128, ot * TT:(ot + 1) * TT], yt)

    nc.compile()
    return nc


_NC_CACHE = None


def _get_nc():
    global _NC_CACHE
    if _NC_CACHE is None:
        _NC_CACHE = build_nc()
    return _NC_CACHE


def make_in_maps(x, W_qkv, W_out):
    x = np.ascontiguousarray(np.asarray(x, dtype=np.float32))
    W_qkv = np.ascontiguousarray(np.asarray(W_qkv, dtype=np.float32))
    W_out = np.ascontiguousarray(np.asarray(W_out, dtype=np.float32))
    xT = [np.ascontiguousarray(x[b].T) for b in range(B)]
    in_maps = []
    for c in range(NCORES):
        b, g = c // 4, c % 4
        rq = W_qkv[g * 256:(g + 1) * 256]            # q rows, heads 4g..4g+3
        rk = W_qkv[C + g * 256:C + (g + 1) * 256]    # k rows
        rv = W_qkv[2 * C + g * 256:2 * C + (g + 1) * 256]  # v rows
        wqkvT = np.ascontiguousarray(np.concatenate([rq, rk, rv], axis=0).T)
        woutT = np.ascontiguousarray(W_out[:, g * 256:(g + 1) * 256].T)
        in_maps.append({"xT": xT[b], "wqkvT": wqkvT, "woutT": woutT})
    return in_maps


def kernel(x, W_qkv, W_out):
    nc = _get_nc()
    in_maps = make_in_maps(x, W_qkv, W_out)
    res = run_bass_kernel_spmd(nc, in_maps, core_ids=list(range(NCORES)))
    kernel.last_results = res
    y = np.zeros((B, T, C), dtype=np.float32)
    for c in range(NCORES):
        y[c // 4] += res.results[c]["y"]
    return y



# revision 2
# speedup vs baseline: 1.3379x; 1.3379x over previous
"""Causal self-attention (B=2, T=2048, D=1024, H=16) on 8 trn2 cores.

Sharding: tensor-parallel over heads x data-parallel over batch.
Core c handles batch b = c // 4, head group g = c % 4 (heads 4g..4g+3).
Host pre-slices/pre-transposes weight+activation shards; each core
returns a partial y (its heads' contribution); host sums groups of 4.

All matmul operands are bf16 (fp32 psum accumulation) — f32r matmuls
lower to fp32_mode=HIGH and trip the PE's 50%-utilization DVFS
throttle, roughly doubling stream time.
"""

import os
import sys

for _p in ("/opt/trn_rl_repo", "/root/.axon_site/_ro/trn_rl_repo"):
    if os.path.isdir(_p) and _p not in sys.path:
        sys.path.insert(0, _p)

import ml_dtypes
import numpy as np

import concourse.bass as bass
import concourse.mybir as mybir
import concourse.tile as tile
from concourse import bacc
from concourse.bass_utils import run_bass_kernel_spmd

F32 = mybir.dt.float32
F32R = mybir.dt.float32r
BF16 = mybir.dt.bfloat16

B, T, C = 2, 2048, 1024
NHEAD_TOT = 16
DH = 64
NCORES = 8
NH = 4          # heads per core
NPAIR = 2       # head pairs per core
CK = C // 128   # contraction chunks (8)
TT = 512        # attention t-tile width
NTT = T // TT   # 4
NSCH = T // 128  # s chunks (16)
FQK = 2 * NH * DH  # 512 cols of qkv^T for q+k
FV = NH * DH       # 256 cols for v


def build_nc(dbg=False):
    nc = bacc.Bacc("TRN2", target_bir_lowering=False, debug=False)

    xT = nc.dram_tensor("xT", [C, T], BF16, kind="ExternalInput")
    wqkvT = nc.dram_tensor("wqkvT", [C, FQK + FV], BF16, kind="ExternalInput")
    woutT = nc.dram_tensor("woutT", [NH * DH, C], BF16, kind="ExternalInput")
    y = nc.dram_tensor("y", [T, C], F32, kind="ExternalOutput")
    L_dram = nc.dram_tensor("L_scratch", [2 * NPAIR * NTT, TT], F32)
    R_dram = nc.dram_tensor("R_scratch", [2 * NPAIR * NTT, TT], F32R)

    EXP = mybir.ActivationFunctionType.Exp

    with tile.TileContext(nc) as tc:
        with (
            tc.tile_pool(name="const", bufs=1) as const,
            tc.tile_pool(name="ptp", bufs=4) as ptp,
            tc.tile_pool(name="bcp", bufs=2) as bcp,
            tc.tile_pool(name="rcp", bufs=2) as rcp,
            tc.tile_pool(name="yp", bufs=2) as yp,
            tc.tile_pool(name="psA", bufs=2, space="PSUM") as psA,
            tc.tile_pool(name="psV", bufs=4, space="PSUM") as psV,
        ):
            # ---- persistent SBUF ----
            xT_sb = const.tile([128, CK, T], BF16)          # x^T  (c-major)
            wqkvT_sb = const.tile([128, CK, FQK + FV], BF16)  # W_qkv^T cols [q(4x64)|k(4x64)|v(4x64)]
            woutT_sb = const.tile([128, NPAIR, C], BF16)    # W_out^T rows per head pair
            qkT_sb = const.tile([128, 4, T], BF16)          # [qPair0|qPair1|kPair0|kPair1] x T
            v_sb = const.tile([128, NSCH, NH, DH + 1], BF16)  # V (s-major) + ones column
            oT_sb = const.tile([128, NPAIR, T], BF16)       # normalized O^T, pair-stacked

            for ci in range(CK):
                nc.sync.dma_start(xT_sb[:, ci, :], xT[ci * 128:(ci + 1) * 128, :])
                nc.sync.dma_start(wqkvT_sb[:, ci, :], wqkvT[ci * 128:(ci + 1) * 128, :])
            for pr in range(NPAIR):
                nc.sync.dma_start(woutT_sb[:, pr, :], woutT[pr * 128:(pr + 1) * 128, :])
            # 1.0 bit pattern for the bf16 ones column
            nc.vector.memset(v_sb[:, :, :, DH:DH + 1].bitcast(mybir.dt.uint16),
                             0x3F80)
            ones1 = const.tile([1, 64], F32R)  # lhsT for recipL row->partition broadcast
            nc.vector.memset(ones1.bitcast(mybir.dt.uint32), 0x3F800000)

            # ---- QKV projection ----
            # q^T/k^T: psum[f128, t512] = sum_c wqkvT[c, f].T @ xT[c, t]
            for ft in range(4):
                for tt in range(NTT):
                    ps = psA.tile([128, 2, TT], F32)
                    for ci in range(CK):
                        nc.tensor.matmul(
                            ps[:, 0, :],
                            wqkvT_sb[:, ci, ft * 128:(ft + 1) * 128],
                            xT_sb[:, ci, tt * TT:(tt + 1) * TT],
                            start=(ci == 0), stop=(ci == CK - 1),
                        )
                    nc.vector.tensor_copy(qkT_sb[:, ft, tt * TT:(tt + 1) * TT], ps[:, 0, :])
            # v natural: psum[t128, f256] = xT[c, t].T @ wqkvT[c, v]
            for si in range(NSCH):
                ps = psA.tile([128, 2, TT], F32)
                for ci in range(CK):
                    nc.tensor.matmul(
                        ps[:, 0, 0:FV],
                        xT_sb[:, ci, si * 128:(si + 1) * 128],
                        wqkvT_sb[:, ci, FQK:FQK + FV],
                        start=(ci == 0), stop=(ci == CK - 1),
                    )
                nc.vector.tensor_copy(
                    v_sb[:, si, :, 0:DH],
                    ps[:, 0, 0:FV].rearrange("p (h d) -> p h d", h=NH),
                )

            # ---- attention (S^T orientation), per head pair ----
            # oT_sb holds UNNORMALIZED O^T during the loop; softmax sums (L)
            # are shipped to DRAM and normalization happens in a batched end
            # phase so PV psums are released by plain copies (no recip chain).
            for pr in range(NPAIR):
                for tt in range(NTT):
                    n_ss = 4 * (tt + 1)  # causal: s-chunks 0 .. 4*tt+3
                    pv = [psV.tile([DH + 1, TT], F32, tag="pv", name=f"pv{pr}_{tt}_{k}")
                          for k in range(2)]
                    for sq in range(n_ss // 2):
                        for hi in range(2):
                            h = pr * 2 + hi
                            ps = psA.tile([128, 2, TT], F32)
                            for i in range(2):
                                ss = 2 * sq + i
                                nc.tensor.matmul(
                                    ps[:, i, :],
                                    qkT_sb[hi * 64:(hi + 1) * 64, 2 + pr, ss * 128:(ss + 1) * 128],
                                    qkT_sb[hi * 64:(hi + 1) * 64, pr, tt * TT:(tt + 1) * TT],
                                )
                            pt = ptp.tile([128, 2, TT], BF16)
                            nc.scalar.activation(pt, ps, EXP, scale=0.125)
                            if 2 * sq >= 4 * tt:  # diagonal quad: zero where s > t
                                nc.gpsimd.affine_select(
                                    out=pt, in_=pt,
                                    compare_op=mybir.AluOpType.is_ge,
                                    fill=0.0,
                                    base=tt * TT - 2 * sq * 128,
                                    channel_multiplier=-1,
                                    pattern=[[-128, 2], [1, TT]],
                                )
                            for i in range(2):
                                ss = 2 * sq + i
                                nc.tensor.matmul(
                                    pv[hi],
                                    v_sb[:, ss, h, :],
                                    pt[:, i, :],
                                    start=(ss == 0), stop=(ss == n_ss - 1),
                                )
                    for hi in range(2):
                        idx = (pr * NTT + tt) * 2 + hi
                        nc.vector.tensor_copy(
                            oT_sb[hi * 64:(hi + 1) * 64, pr, tt * TT:(tt + 1) * TT],
                            pv[hi][0:DH, :],
                        )
                        lrow = rcp.tile([1, TT], F32, tag="lrow", name=f"lrow{idx}")
                        nc.vector.tensor_copy(lrow, pv[hi][DH:DH + 1, :])
                        nc.sync.dma_start(L_dram[idx:idx + 1, :], lrow[0:1, :])

            # ---- batched softmax normalization ----
            # gather all 16 L rows as [128, 64], one fast reciprocal, ship
            # back, then per-tile outer-product broadcast + multiply.
            lsq = bcp.tile([128, 64], F32, tag="lsq")
            nc.sync.dma_start(lsq, L_dram[:, :].rearrange("r (s j) -> (r s) j", j=64))
            with nc.allow_low_precision("f32r recip feeds f32r matmul rhs"):
                rsq = bcp.tile([128, 64], F32R, tag="rsq")
                nc.vector.reciprocal(rsq, lsq)
            nc.sync.dma_start(R_dram[:, :].rearrange("r (s j) -> (r s) j", j=64), rsq)
            for pr in range(NPAIR):
                for tt in range(NTT):
                    bq = psA.tile([128, 2, TT], F32, tag="ps", name=f"bq{pr}_{tt}")
                    bc = bcp.tile([128, TT], BF16)
                    for hi in range(2):
                        idx = (pr * NTT + tt) * 2 + hi
                        rcr = rcp.tile([1, TT], F32R, tag="rcr", name=f"rcr{idx}")
                        nc.sync.dma_start(rcr[0:1, :], R_dram[idx:idx + 1, :])
                        nc.tensor.matmul(bq[:, hi, :][0:64, :], ones1, rcr)
                        nc.vector.tensor_copy(
                            bc[hi * 64:(hi + 1) * 64, :], bq[:, hi, :][0:64, :]
                        )
                        nc.vector.tensor_mul(
                            oT_sb[hi * 64:(hi + 1) * 64, pr, tt * TT:(tt + 1) * TT],
                            oT_sb[hi * 64:(hi + 1) * 64, pr, tt * TT:(tt + 1) * TT],
                            bc[hi * 64:(hi + 1) * 64, :],
                        )

            # ---- output projection: y[t, o] = sum_pr oT[d, t].T @ woutT[d, o] ----
            for tq in range(T // 128):
                for ot in range(C // TT):
                    ps = psA.tile([128, 2, TT], F32)
                    for pr in range(NPAIR):
                        nc.tensor.matmul(
                            ps[:, 0, :],
                            oT_sb[:, pr, tq * 128:(tq + 1) * 128],
                            woutT_sb[:, pr, ot * TT:(ot + 1) * TT],
                            start=(pr == 0), stop=(pr == NPAIR - 1),
                        )
                    yt = yp.tile([128, TT], F32)
                    nc.vector.tensor_copy(yt, ps[:, 0, :])
                    nc.sync.dma_start(y[tq * 128:(tq + 1) * 128, ot * TT:(ot + 1) * TT], yt)

    nc.compile()
    return nc


_NC_CACHE = None


def _get_nc():
    global _NC_CACHE
    if _NC_CACHE is None:
        _NC_CACHE = build_nc()
    return _NC_CACHE


def make_in_maps(x, W_qkv, W_out):
    x = np.ascontiguousarray(np.asarray(x, dtype=np.float32))
    W_qkv = np.ascontiguousarray(np.asarray(W_qkv, dtype=np.float32))
    W_out = np.ascontiguousarray(np.asarray(W_out, dtype=np.float32))
    bf16 = ml_dtypes.bfloat16
    xT = [np.ascontiguousarray(x[b].T.astype(bf16)) for b in range(B)]
    in_maps = []
    for c in range(NCORES):
        b, g = c // 4, c % 4
        rq = W_qkv[g * 256:(g + 1) * 256]            # q rows, heads 4g..4g+3
        rk = W_qkv[C + g * 256:C + (g + 1) * 256]    # k rows
        rv = W_qkv[2 * C + g * 256:2 * C + (g + 1) * 256]  # v rows
        wqkvT = np.ascontiguousarray(
            np.concatenate([rq, rk, rv], axis=0).T.astype(bf16))
        woutT = np.ascontiguousarray(
            W_out[:, g * 256:(g + 1) * 256].T.astype(bf16))
        in_maps.append({"xT": xT[b], "wqkvT": wqkvT, "woutT": woutT})
    return in_maps


def kernel(x, W_qkv, W_out):
    nc = _get_nc()
    in_maps = make_in_maps(x, W_qkv, W_out)
    res = run_bass_kernel_spmd(nc, in_maps, core_ids=list(range(NCORES)))
    kernel.last_results = res
    y = np.zeros((B, T, C), dtype=np.float32)
    for c in range(NCORES):
        y[c // 4] += res.results[c]["y"]
    return y
